# revision 28
# baseline (speedup 1.0000x reference)
"""Trainium2 Bass kernel for nn_CausalMultiresConv1d.

Everything before the final GELU is linear: the whole multires stack is
one combined causal FIR filter per channel, F[c, 0:766], computed on the
host as the impulse response of the reference's linear part.

    out[b, c, n] = gelu( sum_tau F[c, tau] * x[b, c, n - tau] )

Sharding: pure data parallel - 1 batch element per NeuronCore (B=8).

Per-core algorithm (transposed layout so the conv is a PE matmul):
  xt[p, c*cw + 2*(mh + t) + h] = x[c, 16384*h + 128*t + p]        (host)
  i.e. positions-within-block on partitions; channel-major columns with
  (block t, half h) interleaved and mh private halo block-pairs per
  channel (zeros for half 0, the tail of half 0 for half 1), so every
  matmul rhs is one CONTIGUOUS window - strided PE/ACT access patterns
  measured ~3-4x slower.

  For each channel, the FIR becomes M_c banded matmuls accumulated in
  PSUM:   Y_c[p, (t,h)] = sum_m A_m^c.T @ xt[:, window shifted by m]
  with A_m^c[q, p] = F[c, p - q + 128 m]  (128x128 Toeplitz bands, bf16).
  M_c is per-channel via po-averaged tail energy (total truncation error
  ~4e-3 relative incl. bf16, tolerance is 2e-2).

  ACT drains PSUM with exact GELU (contiguous, channel-major); PE
  back-transposes each [128,128] tile (pipelined 2 channels behind the
  conv); DVE drains the transposed tiles bitcast-as-fp32; one DMA ships
  the bf16 result, and the host upcasts + un-interleaves rows.
"""

import numpy as np
import ml_dtypes

import concourse.bass as bass
import concourse.mybir as mybir
from concourse.bass_utils import run_bass_kernel_spmd
from concourse.tile import TileContext

# The walrus build here rejects instructions carrying more than one sync-wait
# ("Too many sync wait commands"). Tile's kernel-tail drain attaches a wait for
# every outstanding semaphore to a single SP Drain. _TC splits them: hoist all
# but the last wait onto dedicated single-wait NOPs preceding the drain.


class _TC(TileContext):
    def __exit__(self, *a):
        r = super().__exit__(*a)
        _split_multi_waits(self.nc)
        return r


def _split_multi_waits(nc):
    n = 0
    for fn in nc.m.functions:
        for blk in fn.blocks:
            insts = getattr(blk, "instructions", None)
            if insts is None:
                continue
            new = []
            for inst in insts:
                si = getattr(inst, "sync_info", None)
                waits = list(si.on_wait) if si is not None and si.on_wait else []
                if len(waits) > 1:
                    for j, wcmd in enumerate(waits[:-1]):
                        nop = mybir.InstNoOp(
                            name=f"{inst.name}-hw{j}", engine=inst.engine
                        )
                        nop.sync_info = mybir.SyncInfo(
                            on_wait=[wcmd], on_update=[]
                        )
                        new.append(nop)
                        n += 1
                    inst.sync_info = mybir.SyncInfo(
                        on_wait=[waits[-1]], on_update=list(si.on_update)
                    )
                new.append(inst)
            blk.instructions[:] = new
    return n


B, C, L = 8, 64, 32768
K, DEPTH = 4, 8
NCORES = 8
NH = 2                  # L-halves packed side by side in the channel dim
HL = L // NH            # 16384 positions per half
NB = HL // 128          # 128 blocks of 128 positions per half
P = 128
FLEN = 766              # combined filter support
MAXM = 7                # max 128-tap bands (covers 766 taps)
TRUNC_THR = 1e-6        # per-channel tail energy cutoff (frac of total)
TSEG = 8                # output blocks per PSUM segment (one 2KB bank)

F32 = mybir.dt.float32
BF16 = mybir.dt.bfloat16


def _combined_filter(h0, h1, w):
    """Impulse response [C, FLEN] of the linear part, in float64."""
    h0d = h0[:, 0, :].astype(np.float64)
    h1d = h1[:, 0, :].astype(np.float64)
    wd = w.astype(np.float64)

    def dconv(r, h, d):
        out = np.zeros_like(r)
        for k in range(K):
            s = (K - 1 - k) * d
            out[:, s:] += h[:, k:k + 1] * r[:, :FLEN - s]
        return out

    r = np.zeros((C, FLEN))
    r[:, 0] = 1.0
    y = np.zeros((C, FLEN))
    d = 1
    for i in range(DEPTH, 0, -1):
        y += wd[:, i][:, None] * dconv(r, h1d, d)
        r = dconv(r, h0d, d)
        d *= 2
    y += wd[:, 0][:, None] * r
    y[:, 0] += wd[:, -1]
    return y


def _choose_mc(F):
    """Per-channel band count. With nb bands, output position po of a block
    sees taps <= po + 128*(nb-1), so the truncation error is the po-AVERAGED
    dropped tail energy; pick the smallest nb that makes it negligible."""
    E = F * F
    tot = E.sum()
    # suffix[t] = sum of E[c, t:]
    suf = np.zeros((C, FLEN + 1))
    suf[:, :FLEN] = E[:, ::-1].cumsum(axis=1)[:, ::-1]
    mc = []
    for c in range(C):
        nb = MAXM
        for M in range(1, MAXM):
            lo = 128 * (M - 1)
            idx = np.minimum(lo + 1 + np.arange(128), FLEN)
            if suf[c, idx].mean() <= TRUNC_THR * tot:
                nb = M
                break
        mc.append(nb)
    return tuple(mc)


def _build_nc(mc, reps=1):
    nc = bass.Bass()
    mh = max(mc) - 1                      # halo blocks
    cw = 2 * mh + 2 * NB                  # columns per channel (halo + data)
    xt_cols = C * cw
    na = sum(mc) + 1                      # band matrices + identity
    xt_in = nc.dram_tensor("xt", [P, xt_cols], BF16, kind="ExternalInput")
    am_in = nc.dram_tensor("am", [P, na * 128], BF16, kind="ExternalInput")
    y_out = nc.dram_tensor("y", [P, HL], BF16, kind="ExternalOutput")

    GELU = mybir.ActivationFunctionType.Gelu

    with _TC(nc) as tc:
        with (
            tc.tile_pool(name="main", bufs=1) as pool,
            tc.tile_pool(name="psum", bufs=1, space="PSUM") as psum_pool,
        ):
            xts = pool.tile([P, xt_cols], BF16, tag="xts")
            ams = pool.tile([P, na * 128], BF16, tag="ams")
            tty = pool.tile([P, NB * 128], BF16, tag="tty")
            ynat = pool.tile([P, NB * 128], BF16, tag="ynat")

            nc.sync.dma_start(out=xts[:], in_=xt_in[:])
            nc.sync.dma_start(out=ams[:], in_=am_in[:])

            ident = ams[:, (na - 1) * 128: na * 128]

            def emit_bt(c0):
                # back-transpose channels c0, c0+1 (four 128-col tiles into
                # one PSUM tile, one DVE drain). Output rows become the
                # (2t+h) interleave that the host unpack untangles; the
                # drain moves bf16 pairs bitcast as fp32 to halve DVE work.
                psb = psum_pool.tile([P, 512], BF16, tag="psb", bufs=2)
                for g in range(4):
                    src = c0 * 256 + g * 128
                    nc.tensor.transpose(
                        psb[:, g * 128: (g + 1) * 128],
                        tty[:, src: src + 128],
                        ident,
                    )
                dst = ynat[:, c0 * 256: (c0 + 2) * 256]
                nc.vector.tensor_copy(dst.bitcast(F32), psb[:].bitcast(F32))

            def emit_body():
                # conv: per channel, mc[c] banded matmuls accumulated in
                # PSUM. xt is channel-major with (t, h)-interleaved columns
                # and a private halo per channel, so every rhs is one
                # CONTIGUOUS 256-column window (strided rhs runs ~3x slower
                # on the PE's SBUF read path).
                off = 0
                for c in range(C):
                    ps = psum_pool.tile([P, 128, 2], F32, tag="ps", bufs=6)
                    for m in range(mc[c]):
                        s0 = c * cw + 2 * (mh - m)
                        nc.tensor.matmul(
                            ps[:],
                            lhsT=ams[:, (off + m) * 128: (off + m + 1) * 128],
                            rhs=xts[:, s0: s0 + 2 * NB],
                            start=(m == 0),
                            stop=(m == mc[c] - 1),
                        )
                    off += mc[c]
                    # exact GELU while draining PSUM; tty is CHANNEL-major
                    # (columns c*256 + 2t + h) so this write is contiguous -
                    # strided engine writes run ~4x slower
                    nc.scalar.activation(
                        out=tty[:, c * 256: (c + 1) * 256],
                        in_=ps.rearrange("p a b -> p (a b)"),
                        func=GELU,
                    )

                    # emit the back-transpose of pair (c-3, c-2) here: two
                    # channels of delay keep the PE queue from head-of-line
                    # blocking on this pair's gelu drains
                    if c >= 3 and c % 2 == 1:
                        emit_bt(c - 3)
                emit_bt(C - 2)

            if reps == 1:
                emit_body()
            else:
                # in-NEFF rep loop for delta timing: constant instruction
                # count, so huge rep counts stay cheap to compile
                with tc.For_i(0, reps):
                    emit_body()
            nc.sync.dma_start(out=y_out[:], in_=ynat[:])
    return nc


_NC_CACHE = {}


def _get_nc(mc, reps=1):
    key = (mc, reps)
    if key not in _NC_CACHE:
        _NC_CACHE[key] = _build_nc(mc, reps)
    return _NC_CACHE[key]


def _band_matrices(F, mc):
    """[P, (sum(mc)+1)*128] bf16: per-channel Toeplitz bands + identity."""
    na = sum(mc) + 1
    am = np.zeros((P, na * 128), np.float32)
    q = np.arange(128)
    off = 0
    for c in range(C):
        Fz = np.zeros(127 + 128 * MAXM + 128)
        Fz[127: 127 + FLEN] = F[c]
        win = np.lib.stride_tricks.sliding_window_view(Fz, 128)
        for m in range(mc[c]):
            # A[q, p] = F[c, p - q + 128 m]
            am[:, (off + m) * 128: (off + m + 1) * 128] = win[127 + 128 * m - q]
        off += mc[c]
    am[:, (na - 1) * 128: na * 128] = np.eye(128, dtype=np.float32)
    return am.astype(ml_dtypes.bfloat16)


def pack_inputs(x, h0, h1, w):
    F = _combined_filter(h0, h1, w)
    mc = _choose_mc(F)
    mh = max(mc) - 1
    am = _band_matrices(F, mc)

    in_maps = []
    for bi in range(NCORES):
        xr = np.ascontiguousarray(x[bi]).reshape(C, NH, NB, 128)
        # channel-major, (t, h)-interleaved columns with per-channel halo:
        # xt[p, c*cw + 2*(mh + t) + h] = x[bi, c, 16384*h + 128*t + p]
        full = np.zeros((P, C, mh + NB, NH), np.float32)
        full[:, :, mh:, :] = xr.transpose(3, 0, 2, 1)
        # half 1's causal history is half 0's last mh blocks
        full[:, :, :mh, 1] = xr[:, 0, NB - mh:, :].transpose(2, 0, 1)
        xt = full.reshape(P, C * (mh + NB) * NH).astype(ml_dtypes.bfloat16)
        in_maps.append({"xt": xt, "am": am})
    return in_maps, mc


def unpack_outputs(results):
    # device output rows are the (2*t + h) interleave of each transposed
    # channel-major tile; columns are (c, tile-half g, p)
    out = np.empty((B, C, L), np.float32)
    for bi, r in enumerate(results):
        yv = np.asarray(r["y"]).astype(np.float32)
        v = yv.reshape(64, NH, C, 2, 128)            # [tm, h, c, g, p]
        out[bi] = v.transpose(2, 1, 3, 0, 4).reshape(C, L)
    return out


def kernel(x, h0, h1, w, _trace=False):
    import os
    os.environ.setdefault("BASS_NEVER_TRACE", "1")

    x = np.asarray(x, np.float32)
    h0 = np.asarray(h0, np.float32)
    h1 = np.asarray(h1, np.float32)
    w = np.asarray(w, np.float32)

    in_maps, mc = pack_inputs(x, h0, h1, w)
    nc = _get_nc(mc, 1)
    try:
        res = run_bass_kernel_spmd(
            nc, in_maps, core_ids=list(range(NCORES)), trace=_trace,
        )
    except Exception:
        # transient "device unrecoverable" failures have been observed on
        # this fleet; one retry usually succeeds
        res = run_bass_kernel_spmd(
            nc, in_maps, core_ids=list(range(NCORES)), trace=_trace,
        )
    out = unpack_outputs(res.results)
    if _trace:
        return out, res
    return out


# revision 29
# speedup vs baseline: 1.0481x; 1.0481x over previous
"""Trainium2 Bass kernel for nn_CausalMultiresConv1d.

Everything before the final GELU is linear: the whole multires stack is
one combined causal FIR filter per channel, F[c, 0:766], computed on the
host as the impulse response of the reference's linear part.

    out[b, c, n] = gelu( sum_tau F[c, tau] * x[b, c, n - tau] )

Sharding: pure data parallel - 1 batch element per NeuronCore (B=8).

Per-core algorithm (transposed layout so the conv is a PE matmul):
  xt[p, c*cw + 2*(mh + t) + h] = x[c, 16384*h + 128*t + p]        (host)
  i.e. positions-within-block on partitions; channel-major columns with
  (block t, half h) interleaved and mh private halo block-pairs per
  channel (zeros for half 0, the tail of half 0 for half 1), so every
  matmul rhs is one CONTIGUOUS window - strided PE/ACT access patterns
  measured ~3-4x slower.

  For each channel, the FIR becomes M_c banded matmuls accumulated in
  PSUM:   Y_c[p, (t,h)] = sum_m A_m^c.T @ xt[:, window shifted by m]
  with A_m^c[q, p] = F[c, p - q + 128 m]  (128x128 Toeplitz bands, bf16).
  M_c is per-channel via po-averaged tail energy (total truncation error
  ~4e-3 relative incl. bf16, tolerance is 2e-2).

  ACT drains PSUM with exact GELU (contiguous, channel-major); PE
  back-transposes each [128,128] tile (pipelined 2 channels behind the
  conv); DVE drains the transposed tiles bitcast-as-fp32; one DMA ships
  the bf16 result, and the host upcasts + un-interleaves rows.
"""

import numpy as np
import ml_dtypes

import concourse.bass as bass
import concourse.mybir as mybir
from concourse.bass_utils import run_bass_kernel_spmd
from concourse.tile import TileContext

# The walrus build here rejects instructions carrying more than one sync-wait
# ("Too many sync wait commands"). Tile's kernel-tail drain attaches a wait for
# every outstanding semaphore to a single SP Drain. _TC splits them: hoist all
# but the last wait onto dedicated single-wait NOPs preceding the drain.


class _TC(TileContext):
    def __exit__(self, *a):
        r = super().__exit__(*a)
        _split_multi_waits(self.nc)
        return r


def _split_multi_waits(nc):
    n = 0
    for fn in nc.m.functions:
        for blk in fn.blocks:
            insts = getattr(blk, "instructions", None)
            if insts is None:
                continue
            new = []
            for inst in insts:
                si = getattr(inst, "sync_info", None)
                waits = list(si.on_wait) if si is not None and si.on_wait else []
                if len(waits) > 1:
                    for j, wcmd in enumerate(waits[:-1]):
                        nop = mybir.InstNoOp(
                            name=f"{inst.name}-hw{j}", engine=inst.engine
                        )
                        nop.sync_info = mybir.SyncInfo(
                            on_wait=[wcmd], on_update=[]
                        )
                        new.append(nop)
                        n += 1
                    inst.sync_info = mybir.SyncInfo(
                        on_wait=[waits[-1]], on_update=list(si.on_update)
                    )
                new.append(inst)
            blk.instructions[:] = new
    return n


B, C, L = 8, 64, 32768
K, DEPTH = 4, 8
NCORES = 8
NH = 2                  # L-halves packed side by side in the channel dim
HL = L // NH            # 16384 positions per half
NB = HL // 128          # 128 blocks of 128 positions per half
P = 128
FLEN = 766              # combined filter support
MAXM = 7                # max 128-tap bands (covers 766 taps)
TRUNC_THR = 1e-6        # per-channel tail energy cutoff (frac of total)
TSEG = 8                # output blocks per PSUM segment (one 2KB bank)

F32 = mybir.dt.float32
BF16 = mybir.dt.bfloat16


def _combined_filter(h0, h1, w):
    """Impulse response [C, FLEN] of the linear part, in float64."""
    h0d = h0[:, 0, :].astype(np.float64)
    h1d = h1[:, 0, :].astype(np.float64)
    wd = w.astype(np.float64)

    def dconv(r, h, d):
        out = np.zeros_like(r)
        for k in range(K):
            s = (K - 1 - k) * d
            out[:, s:] += h[:, k:k + 1] * r[:, :FLEN - s]
        return out

    r = np.zeros((C, FLEN))
    r[:, 0] = 1.0
    y = np.zeros((C, FLEN))
    d = 1
    for i in range(DEPTH, 0, -1):
        y += wd[:, i][:, None] * dconv(r, h1d, d)
        r = dconv(r, h0d, d)
        d *= 2
    y += wd[:, 0][:, None] * r
    y[:, 0] += wd[:, -1]
    return y


def _choose_mc(F):
    """Per-channel band count. With nb bands, output position po of a block
    sees taps <= po + 128*(nb-1), so the truncation error is the po-AVERAGED
    dropped tail energy; pick the smallest nb that makes it negligible."""
    E = F * F
    tot = E.sum()
    # suffix[t] = sum of E[c, t:]
    suf = np.zeros((C, FLEN + 1))
    suf[:, :FLEN] = E[:, ::-1].cumsum(axis=1)[:, ::-1]
    mc = []
    for c in range(C):
        nb = MAXM
        for M in range(1, MAXM):
            lo = 128 * (M - 1)
            idx = np.minimum(lo + 1 + np.arange(128), FLEN)
            if suf[c, idx].mean() <= TRUNC_THR * tot:
                nb = M
                break
        mc.append(nb)
    return tuple(mc)


def _build_nc(mc, reps=1):
    nc = bass.Bass()
    mh = max(mc) - 1                      # halo blocks
    cw = 2 * mh + 2 * NB                  # columns per channel (halo + data)
    xt_cols = C * cw
    na = sum(mc) + 1                      # band matrices + identity
    xt_in = nc.dram_tensor("xt", [P, xt_cols], BF16, kind="ExternalInput")
    am_in = nc.dram_tensor("am", [P, na * 128], BF16, kind="ExternalInput")
    y_out = nc.dram_tensor("y", [P, HL], BF16, kind="ExternalOutput")

    GELU = mybir.ActivationFunctionType.Gelu

    with _TC(nc) as tc:
        with (
            tc.tile_pool(name="main", bufs=1) as pool,
            tc.tile_pool(name="psum", bufs=1, space="PSUM") as psum_pool,
        ):
            xts = pool.tile([P, xt_cols], BF16, tag="xts")
            ams = pool.tile([P, na * 128], BF16, tag="ams")
            tty = pool.tile([P, NB * 128], BF16, tag="tty")
            ynat = pool.tile([P, NB * 128], BF16, tag="ynat")

            nc.sync.dma_start(out=xts[:], in_=xt_in[:])
            nc.sync.dma_start(out=ams[:], in_=am_in[:])

            ident = ams[:, (na - 1) * 128: na * 128]

            def emit_bt(c0):
                # back-transpose channels c0, c0+1 (four 128-col tiles into
                # one PSUM tile, one DVE drain). Output rows become the
                # (2t+h) interleave that the host unpack untangles; the
                # drain moves bf16 pairs bitcast as fp32 to halve DVE work.
                psb = psum_pool.tile([P, 512], BF16, tag="psb", bufs=2)
                for g in range(4):
                    src = c0 * 256 + g * 128
                    nc.tensor.transpose(
                        psb[:, g * 128: (g + 1) * 128],
                        tty[:, src: src + 128],
                        ident,
                    )
                dst = ynat[:, c0 * 256: (c0 + 2) * 256]
                nc.vector.tensor_copy(dst.bitcast(F32), psb[:].bitcast(F32))

            def emit_body():
                # conv: per channel, mc[c] banded matmuls accumulated in
                # PSUM. xt is channel-major with (t, h)-interleaved columns
                # and a private halo per channel, so every rhs is one
                # CONTIGUOUS 256-column window (strided rhs runs ~3x slower
                # on the PE's SBUF read path). Channels are processed in
                # pairs sharing one PSUM bank so each gelu drain covers 512
                # columns - ACT per-instruction overhead is a co-bottleneck.
                off = 0
                for k in range(C // 2):
                    ps = psum_pool.tile([P, 2, 128, 2], F32, tag="ps", bufs=3)
                    for ci in range(2):
                        c = 2 * k + ci
                        for m in range(mc[c]):
                            s0 = c * cw + 2 * (mh - m)
                            nc.tensor.matmul(
                                ps[:, ci],
                                lhsT=ams[:, (off + m) * 128:
                                          (off + m + 1) * 128],
                                rhs=xts[:, s0: s0 + 2 * NB],
                                start=(m == 0),
                                stop=(m == mc[c] - 1),
                            )
                        off += mc[c]
                    # exact GELU while draining PSUM; tty is CHANNEL-major
                    # (columns c*256 + 2t + h) so this write is contiguous -
                    # strided engine writes run ~4x slower
                    nc.scalar.activation(
                        out=tty[:, k * 512: (k + 1) * 512],
                        in_=ps.rearrange("p a b h -> p (a b h)"),
                        func=GELU,
                    )

                    # emit the back-transpose of pair k-2 here: two pairs of
                    # delay keep the PE queue from head-of-line blocking on
                    # that pair's gelu drain
                    if k >= 2:
                        emit_bt(2 * (k - 2))
                emit_bt(C - 4)
                emit_bt(C - 2)

            if reps == 1:
                emit_body()
            else:
                # in-NEFF rep loop for delta timing: constant instruction
                # count, so huge rep counts stay cheap to compile
                with tc.For_i(0, reps):
                    emit_body()
            nc.sync.dma_start(out=y_out[:], in_=ynat[:])
    return nc


_NC_CACHE = {}


def _get_nc(mc, reps=1):
    key = (mc, reps)
    if key not in _NC_CACHE:
        _NC_CACHE[key] = _build_nc(mc, reps)
    return _NC_CACHE[key]


def _band_matrices(F, mc):
    """[P, (sum(mc)+1)*128] bf16: per-channel Toeplitz bands + identity."""
    na = sum(mc) + 1
    am = np.zeros((P, na * 128), np.float32)
    q = np.arange(128)
    off = 0
    for c in range(C):
        Fz = np.zeros(127 + 128 * MAXM + 128)
        Fz[127: 127 + FLEN] = F[c]
        win = np.lib.stride_tricks.sliding_window_view(Fz, 128)
        for m in range(mc[c]):
            # A[q, p] = F[c, p - q + 128 m]
            am[:, (off + m) * 128: (off + m + 1) * 128] = win[127 + 128 * m - q]
        off += mc[c]
    am[:, (na - 1) * 128: na * 128] = np.eye(128, dtype=np.float32)
    return am.astype(ml_dtypes.bfloat16)


def pack_inputs(x, h0, h1, w):
    F = _combined_filter(h0, h1, w)
    mc = _choose_mc(F)
    mh = max(mc) - 1
    am = _band_matrices(F, mc)

    in_maps = []
    for bi in range(NCORES):
        xr = np.ascontiguousarray(x[bi]).reshape(C, NH, NB, 128)
        # channel-major, (t, h)-interleaved columns with per-channel halo:
        # xt[p, c*cw + 2*(mh + t) + h] = x[bi, c, 16384*h + 128*t + p]
        full = np.zeros((P, C, mh + NB, NH), np.float32)
        full[:, :, mh:, :] = xr.transpose(3, 0, 2, 1)
        # half 1's causal history is half 0's last mh blocks
        full[:, :, :mh, 1] = xr[:, 0, NB - mh:, :].transpose(2, 0, 1)
        xt = full.reshape(P, C * (mh + NB) * NH).astype(ml_dtypes.bfloat16)
        in_maps.append({"xt": xt, "am": am})
    return in_maps, mc


def unpack_outputs(results):
    # device output rows are the (2*t + h) interleave of each transposed
    # channel-major tile; columns are (c, tile-half g, p)
    out = np.empty((B, C, L), np.float32)
    for bi, r in enumerate(results):
        yv = np.asarray(r["y"]).astype(np.float32)
        v = yv.reshape(64, NH, C, 2, 128)            # [tm, h, c, g, p]
        out[bi] = v.transpose(2, 1, 3, 0, 4).reshape(C, L)
    return out


def kernel(x, h0, h1, w, _trace=False):
    import os
    os.environ.setdefault("BASS_NEVER_TRACE", "1")

    x = np.asarray(x, np.float32)
    h0 = np.asarray(h0, np.float32)
    h1 = np.asarray(h1, np.float32)
    w = np.asarray(w, np.float32)

    in_maps, mc = pack_inputs(x, h0, h1, w)
    nc = _get_nc(mc, 1)
    try:
        res = run_bass_kernel_spmd(
            nc, in_maps, core_ids=list(range(NCORES)), trace=_trace,
        )
    except Exception:
        # transient "device unrecoverable" failures have been observed on
        # this fleet; one retry usually succeeds
        res = run_bass_kernel_spmd(
            nc, in_maps, core_ids=list(range(NCORES)), trace=_trace,
        )
    out = unpack_outputs(res.results)
    if _trace:
        return out, res
    return out


# revision 31
# speedup vs baseline: 1.0903x; 1.0403x over previous
"""Trainium2 Bass kernel for nn_CausalMultiresConv1d.

Everything before the final GELU is linear: the whole multires stack is
one combined causal FIR filter per channel, F[c, 0:766], computed on the
host as the impulse response of the reference's linear part.

    out[b, c, n] = gelu( sum_tau F[c, tau] * x[b, c, n - tau] )

Sharding: pure data parallel - 1 batch element per NeuronCore (B=8).

Per-core algorithm (transposed layout so the conv is a PE matmul):
  xt[p, c*cw + 2*(mh + t) + h] = x[c, 16384*h + 128*t + p]        (host)
  i.e. positions-within-block on partitions; channel-major columns with
  (block t, half h) interleaved and mh private halo block-pairs per
  channel (zeros for half 0, the tail of half 0 for half 1), so every
  matmul rhs is one CONTIGUOUS window - strided PE/ACT access patterns
  measured ~3-4x slower.

  For each channel, the FIR becomes M_c banded matmuls accumulated in
  PSUM:   Y_c[p, (t,h)] = sum_m A_m^c.T @ xt[:, window shifted by m]
  with A_m^c[q, p] = F[c, p - q + 128 m]  (128x128 Toeplitz bands, bf16).
  M_c is per-channel via po-averaged tail energy (total truncation error
  ~4e-3 relative incl. bf16, tolerance is 2e-2).

  ACT drains PSUM with exact GELU (contiguous, channel-major); PE
  back-transposes each [128,128] tile (pipelined 2 channels behind the
  conv); DVE drains the transposed tiles bitcast-as-fp32; one DMA ships
  the bf16 result, and the host upcasts + un-interleaves rows.
"""

import numpy as np
import ml_dtypes

import concourse.bass as bass
import concourse.mybir as mybir
from concourse.bass_utils import run_bass_kernel_spmd
from concourse.tile import TileContext

# The walrus build here rejects instructions carrying more than one sync-wait
# ("Too many sync wait commands"). Tile's kernel-tail drain attaches a wait for
# every outstanding semaphore to a single SP Drain. _TC splits them: hoist all
# but the last wait onto dedicated single-wait NOPs preceding the drain.


class _TC(TileContext):
    def __exit__(self, *a):
        r = super().__exit__(*a)
        _split_multi_waits(self.nc)
        return r


def _split_multi_waits(nc):
    n = 0
    for fn in nc.m.functions:
        for blk in fn.blocks:
            insts = getattr(blk, "instructions", None)
            if insts is None:
                continue
            new = []
            for inst in insts:
                si = getattr(inst, "sync_info", None)
                waits = list(si.on_wait) if si is not None and si.on_wait else []
                if len(waits) > 1:
                    for j, wcmd in enumerate(waits[:-1]):
                        nop = mybir.InstNoOp(
                            name=f"{inst.name}-hw{j}", engine=inst.engine
                        )
                        nop.sync_info = mybir.SyncInfo(
                            on_wait=[wcmd], on_update=[]
                        )
                        new.append(nop)
                        n += 1
                    inst.sync_info = mybir.SyncInfo(
                        on_wait=[waits[-1]], on_update=list(si.on_update)
                    )
                new.append(inst)
            blk.instructions[:] = new
    return n


B, C, L = 8, 64, 32768
K, DEPTH = 4, 8
NCORES = 8
NH = 2                  # L-halves packed side by side in the channel dim
HL = L // NH            # 16384 positions per half
NB = HL // 128          # 128 blocks of 128 positions per half
P = 128
FLEN = 766              # combined filter support
MAXM = 7                # max 128-tap bands (covers 766 taps)
TRUNC_THR = 3e-6        # per-channel tail energy cutoff (frac of total)
TSEG = 8                # output blocks per PSUM segment (one 2KB bank)

F32 = mybir.dt.float32
BF16 = mybir.dt.bfloat16


def _combined_filter(h0, h1, w):
    """Impulse response [C, FLEN] of the linear part, in float64."""
    h0d = h0[:, 0, :].astype(np.float64)
    h1d = h1[:, 0, :].astype(np.float64)
    wd = w.astype(np.float64)

    def dconv(r, h, d):
        out = np.zeros_like(r)
        for k in range(K):
            s = (K - 1 - k) * d
            out[:, s:] += h[:, k:k + 1] * r[:, :FLEN - s]
        return out

    r = np.zeros((C, FLEN))
    r[:, 0] = 1.0
    y = np.zeros((C, FLEN))
    d = 1
    for i in range(DEPTH, 0, -1):
        y += wd[:, i][:, None] * dconv(r, h1d, d)
        r = dconv(r, h0d, d)
        d *= 2
    y += wd[:, 0][:, None] * r
    y[:, 0] += wd[:, -1]
    return y


def _choose_mc(F):
    """Per-channel band count. With nb bands, output position po of a block
    sees taps <= po + 128*(nb-1), so the truncation error is the po-AVERAGED
    dropped tail energy; pick the smallest nb that makes it negligible."""
    E = F * F
    tot = E.sum()
    # suffix[t] = sum of E[c, t:]
    suf = np.zeros((C, FLEN + 1))
    suf[:, :FLEN] = E[:, ::-1].cumsum(axis=1)[:, ::-1]
    mc = []
    for c in range(C):
        nb = MAXM
        for M in range(1, MAXM):
            lo = 128 * (M - 1)
            idx = np.minimum(lo + 1 + np.arange(128), FLEN)
            if suf[c, idx].mean() <= TRUNC_THR * tot:
                nb = M
                break
        mc.append(nb)
    return tuple(mc)


def _build_nc(mc, reps=1):
    nc = bass.Bass()
    mh = max(mc) - 1                      # halo blocks
    cw = 2 * mh + 2 * NB                  # columns per channel (halo + data)
    xt_cols = C * cw
    na = sum(mc) + 1                      # band matrices + identity
    xt_in = nc.dram_tensor("xt", [P, xt_cols], BF16, kind="ExternalInput")
    am_in = nc.dram_tensor("am", [P, na * 128], BF16, kind="ExternalInput")
    y_out = nc.dram_tensor("y", [P, HL], BF16, kind="ExternalOutput")

    GELU = mybir.ActivationFunctionType.Gelu

    with _TC(nc) as tc:
        with (
            tc.tile_pool(name="main", bufs=1) as pool,
            tc.tile_pool(name="psum", bufs=1, space="PSUM") as psum_pool,
        ):
            xts = pool.tile([P, xt_cols], BF16, tag="xts")
            ams = pool.tile([P, na * 128], BF16, tag="ams")
            tty = pool.tile([P, NB * 128], BF16, tag="tty")
            ynat = pool.tile([P, NB * 128], BF16, tag="ynat")

            nc.sync.dma_start(out=xts[:], in_=xt_in[:])
            nc.sync.dma_start(out=ams[:], in_=am_in[:])

            ident = ams[:, (na - 1) * 128: na * 128]

            def emit_bt(c0):
                # back-transpose channels c0, c0+1 (four 128-col tiles into
                # one PSUM tile, one DVE drain). Output rows become the
                # (2t+h) interleave that the host unpack untangles; the
                # drain moves bf16 pairs bitcast as fp32 to halve DVE work.
                psb = psum_pool.tile([P, 512], BF16, tag="psb", bufs=4)
                for g in range(4):
                    src = c0 * 256 + g * 128
                    nc.tensor.transpose(
                        psb[:, g * 128: (g + 1) * 128],
                        tty[:, src: src + 128],
                        ident,
                    )
                dst = ynat[:, c0 * 256: (c0 + 2) * 256]
                nc.vector.tensor_copy(dst.bitcast(F32), psb[:].bitcast(F32))

            def emit_body():
                # conv: per channel, mc[c] banded matmuls accumulated in
                # PSUM. xt is channel-major with (t, h)-interleaved columns
                # and a private halo per channel, so every rhs is one
                # CONTIGUOUS 256-column window (strided rhs runs ~3x slower
                # on the PE's SBUF read path). Channels are processed in
                # pairs sharing one PSUM bank so each gelu drain covers 512
                # columns - ACT per-instruction overhead is a co-bottleneck.
                off = 0
                for k in range(C // 2):
                    ps = psum_pool.tile([P, 2, 128, 2], F32, tag="ps", bufs=4)
                    for ci in range(2):
                        c = 2 * k + ci
                        for m in range(mc[c]):
                            s0 = c * cw + 2 * (mh - m)
                            nc.tensor.matmul(
                                ps[:, ci],
                                lhsT=ams[:, (off + m) * 128:
                                          (off + m + 1) * 128],
                                rhs=xts[:, s0: s0 + 2 * NB],
                                start=(m == 0),
                                stop=(m == mc[c] - 1),
                            )
                        off += mc[c]
                    # exact GELU while draining PSUM; tty is CHANNEL-major
                    # (columns c*256 + 2t + h) so this write is contiguous -
                    # strided engine writes run ~4x slower
                    nc.scalar.activation(
                        out=tty[:, k * 512: (k + 1) * 512],
                        in_=ps.rearrange("p a b h -> p (a b h)"),
                        func=GELU,
                    )

                    # emit the back-transpose of pair k-2 here: two pairs of
                    # delay keep the PE queue from head-of-line blocking on
                    # that pair's gelu drain
                    if k >= 2:
                        emit_bt(2 * (k - 2))
                emit_bt(C - 4)
                emit_bt(C - 2)

            if reps == 1:
                emit_body()
            else:
                # in-NEFF rep loop for delta timing: constant instruction
                # count, so huge rep counts stay cheap to compile
                with tc.For_i(0, reps):
                    emit_body()
            nc.sync.dma_start(out=y_out[:], in_=ynat[:])
    return nc


_NC_CACHE = {}


def _get_nc(mc, reps=1):
    key = (mc, reps)
    if key not in _NC_CACHE:
        _NC_CACHE[key] = _build_nc(mc, reps)
    return _NC_CACHE[key]


def _band_matrices(F, mc):
    """[P, (sum(mc)+1)*128] bf16: per-channel Toeplitz bands + identity."""
    na = sum(mc) + 1
    am = np.zeros((P, na * 128), np.float32)
    q = np.arange(128)
    off = 0
    for c in range(C):
        Fz = np.zeros(127 + 128 * MAXM + 128)
        Fz[127: 127 + FLEN] = F[c]
        win = np.lib.stride_tricks.sliding_window_view(Fz, 128)
        for m in range(mc[c]):
            # A[q, p] = F[c, p - q + 128 m]
            am[:, (off + m) * 128: (off + m + 1) * 128] = win[127 + 128 * m - q]
        off += mc[c]
    am[:, (na - 1) * 128: na * 128] = np.eye(128, dtype=np.float32)
    return am.astype(ml_dtypes.bfloat16)


def pack_inputs(x, h0, h1, w):
    F = _combined_filter(h0, h1, w)
    mc = _choose_mc(F)
    mh = max(mc) - 1
    am = _band_matrices(F, mc)

    in_maps = []
    for bi in range(NCORES):
        xr = np.ascontiguousarray(x[bi]).reshape(C, NH, NB, 128)
        # channel-major, (t, h)-interleaved columns with per-channel halo:
        # xt[p, c*cw + 2*(mh + t) + h] = x[bi, c, 16384*h + 128*t + p]
        full = np.zeros((P, C, mh + NB, NH), np.float32)
        full[:, :, mh:, :] = xr.transpose(3, 0, 2, 1)
        # half 1's causal history is half 0's last mh blocks
        full[:, :, :mh, 1] = xr[:, 0, NB - mh:, :].transpose(2, 0, 1)
        xt = full.reshape(P, C * (mh + NB) * NH).astype(ml_dtypes.bfloat16)
        in_maps.append({"xt": xt, "am": am})
    return in_maps, mc


def unpack_outputs(results):
    # device output rows are the (2*t + h) interleave of each transposed
    # channel-major tile; columns are (c, tile-half g, p)
    out = np.empty((B, C, L), np.float32)
    for bi, r in enumerate(results):
        yv = np.asarray(r["y"]).astype(np.float32)
        v = yv.reshape(64, NH, C, 2, 128)            # [tm, h, c, g, p]
        out[bi] = v.transpose(2, 1, 3, 0, 4).reshape(C, L)
    return out


def kernel(x, h0, h1, w, _trace=False):
    import os
    os.environ.setdefault("BASS_NEVER_TRACE", "1")

    x = np.asarray(x, np.float32)
    h0 = np.asarray(h0, np.float32)
    h1 = np.asarray(h1, np.float32)
    w = np.asarray(w, np.float32)

    in_maps, mc = pack_inputs(x, h0, h1, w)
    nc = _get_nc(mc, 1)
    try:
        res = run_bass_kernel_spmd(
            nc, in_maps, core_ids=list(range(NCORES)), trace=_trace,
        )
    except Exception:
        # transient "device unrecoverable" failures have been observed on
        # this fleet; one retry usually succeeds
        res = run_bass_kernel_spmd(
            nc, in_maps, core_ids=list(range(NCORES)), trace=_trace,
        )
    out = unpack_outputs(res.results)
    if _trace:
        return out, res
    return out


# revision 33
# speedup vs baseline: 1.2046x; 1.1049x over previous
"""Trainium2 Bass kernel for nn_CausalMultiresConv1d.

Everything before the final GELU is linear: the whole multires stack is
one combined causal FIR filter per channel, F[c, 0:766], computed on the
host as the impulse response of the reference's linear part.

    out[b, c, n] = gelu( sum_tau F[c, tau] * x[b, c, n - tau] )

Sharding: pure data parallel - 1 batch element per NeuronCore (B=8).

Per-core algorithm (transposed layout so the conv is a PE matmul):
  xt[p, c*cw + 2*(mh + t) + h] = x[c, 16384*h + 128*t + p]        (host)
  i.e. positions-within-block on partitions; channel-major columns with
  (block t, half h) interleaved and mh private halo block-pairs per
  channel (zeros for half 0, the tail of half 0 for half 1), so every
  matmul rhs is one CONTIGUOUS window - strided PE/ACT access patterns
  measured ~3-4x slower.

  For each channel, the FIR becomes M_c banded matmuls accumulated in
  PSUM:   Y_c[p, (t,h)] = sum_m A_m^c.T @ xt[:, window shifted by m]
  with A_m^c[q, p] = F[c, p - q + 128 m]  (128x128 Toeplitz bands, bf16).
  M_c is per-channel via po-averaged tail energy (total truncation error
  ~4e-3 relative incl. bf16, tolerance is 2e-2).

  ACT drains PSUM with exact GELU (contiguous, channel-major); PE
  back-transposes each [128,128] tile (pipelined 2 channels behind the
  conv); DVE drains the transposed tiles bitcast-as-fp32; one DMA ships
  the bf16 result, and the host upcasts + un-interleaves rows.
"""

import numpy as np
import ml_dtypes

import concourse.bass as bass
import concourse.mybir as mybir
from concourse.bass_utils import run_bass_kernel_spmd
from concourse.tile import TileContext

# The walrus build here rejects instructions carrying more than one sync-wait
# ("Too many sync wait commands"). Tile's kernel-tail drain attaches a wait for
# every outstanding semaphore to a single SP Drain. _TC splits them: hoist all
# but the last wait onto dedicated single-wait NOPs preceding the drain.


class _TC(TileContext):
    def __exit__(self, *a):
        r = super().__exit__(*a)
        _split_multi_waits(self.nc)
        return r


def _split_multi_waits(nc):
    n = 0
    for fn in nc.m.functions:
        for blk in fn.blocks:
            insts = getattr(blk, "instructions", None)
            if insts is None:
                continue
            new = []
            for inst in insts:
                si = getattr(inst, "sync_info", None)
                waits = list(si.on_wait) if si is not None and si.on_wait else []
                if len(waits) > 1:
                    for j, wcmd in enumerate(waits[:-1]):
                        nop = mybir.InstNoOp(
                            name=f"{inst.name}-hw{j}", engine=inst.engine
                        )
                        nop.sync_info = mybir.SyncInfo(
                            on_wait=[wcmd], on_update=[]
                        )
                        new.append(nop)
                        n += 1
                    inst.sync_info = mybir.SyncInfo(
                        on_wait=[waits[-1]], on_update=list(si.on_update)
                    )
                new.append(inst)
            blk.instructions[:] = new
    return n


B, C, L = 8, 64, 32768
K, DEPTH = 4, 8
NCORES = 8
NH = 2                  # L-halves packed side by side in the channel dim
HL = L // NH            # 16384 positions per half
NB = HL // 128          # 128 blocks of 128 positions per half
P = 128
FLEN = 766              # combined filter support
MAXM = 7                # max 128-tap bands (covers 766 taps)
TRUNC_THR = 3e-6        # per-channel tail energy cutoff (frac of total)
TSEG = 8                # output blocks per PSUM segment (one 2KB bank)

F32 = mybir.dt.float32
BF16 = mybir.dt.bfloat16


def _combined_filter(h0, h1, w):
    """Impulse response [C, FLEN] of the linear part, in float64."""
    h0d = h0[:, 0, :].astype(np.float64)
    h1d = h1[:, 0, :].astype(np.float64)
    wd = w.astype(np.float64)

    def dconv(r, h, d):
        out = np.zeros_like(r)
        for k in range(K):
            s = (K - 1 - k) * d
            out[:, s:] += h[:, k:k + 1] * r[:, :FLEN - s]
        return out

    r = np.zeros((C, FLEN))
    r[:, 0] = 1.0
    y = np.zeros((C, FLEN))
    d = 1
    for i in range(DEPTH, 0, -1):
        y += wd[:, i][:, None] * dconv(r, h1d, d)
        r = dconv(r, h0d, d)
        d *= 2
    y += wd[:, 0][:, None] * r
    y[:, 0] += wd[:, -1]
    return y


def _choose_mc(F):
    """Per-channel band count. With nb bands, output position po of a block
    sees taps <= po + 128*(nb-1), so the truncation error is the po-AVERAGED
    dropped tail energy; pick the smallest nb that makes it negligible."""
    E = F * F
    tot = E.sum()
    # suffix[t] = sum of E[c, t:]
    suf = np.zeros((C, FLEN + 1))
    suf[:, :FLEN] = E[:, ::-1].cumsum(axis=1)[:, ::-1]
    mc = []
    for c in range(C):
        nb = MAXM
        for M in range(1, MAXM):
            lo = 128 * (M - 1)
            idx = np.minimum(lo + 1 + np.arange(128), FLEN)
            if suf[c, idx].mean() <= TRUNC_THR * tot:
                nb = M
                break
        mc.append(nb)
    return tuple(mc)


def _build_nc(mc, reps=1):
    nc = bass.Bass()
    mh = max(mc) - 1                      # halo blocks
    cw = 2 * mh + 2 * NB                  # columns per channel (halo + data)
    xt_cols = C * cw
    na = sum(mc) + 1                      # band matrices + identity
    xt_in = nc.dram_tensor("xt", [P, xt_cols], BF16, kind="ExternalInput")
    am_in = nc.dram_tensor("am", [P, na * 128], BF16, kind="ExternalInput")
    y_out = nc.dram_tensor("y", [P, HL], BF16, kind="ExternalOutput")

    GELU = mybir.ActivationFunctionType.Gelu

    with _TC(nc) as tc:
        with (
            tc.tile_pool(name="main", bufs=1) as pool,
            tc.tile_pool(name="psum", bufs=1, space="PSUM") as psum_pool,
        ):
            xts = pool.tile([P, xt_cols], BF16, tag="xts")
            ams = pool.tile([P, na * 128], BF16, tag="ams")
            tty = pool.tile([P, NB * 128], BF16, tag="tty")
            ynat = pool.tile([P, NB * 128], BF16, tag="ynat")

            nc.sync.dma_start(out=xts[:], in_=xt_in[:])
            nc.sync.dma_start(out=ams[:], in_=am_in[:])

            def emit_bt(c0):
                # back-transpose channels c0, c0+1 with DVE StreamTranspose
                # (SBUF->SBUF, 32x32 blocks transposed in place-position).
                # This keeps the whole reorder off the PE and skips the
                # PSUM round-trip; the host unpack absorbs the block-grid
                # permutation (it is a pure numpy axis permute either way).
                src = c0 * 256
                nc.vector.transpose(
                    ynat[:, src: src + 512], tty[:, src: src + 512],
                )

            def emit_body():
                # conv: per channel, mc[c] banded matmuls accumulated in
                # PSUM. xt is channel-major with (t, h)-interleaved columns
                # and a private halo per channel, so every rhs is one
                # CONTIGUOUS 256-column window (strided rhs runs ~3x slower
                # on the PE's SBUF read path). Channels are processed in
                # pairs sharing one PSUM bank so each gelu drain covers 512
                # columns - ACT per-instruction overhead is a co-bottleneck.
                off = 0
                for k in range(C // 2):
                    ps = psum_pool.tile([P, 2, 128, 2], F32, tag="ps", bufs=4)
                    for ci in range(2):
                        c = 2 * k + ci
                        for m in range(mc[c]):
                            s0 = c * cw + 2 * (mh - m)
                            nc.tensor.matmul(
                                ps[:, ci],
                                lhsT=ams[:, (off + m) * 128:
                                          (off + m + 1) * 128],
                                rhs=xts[:, s0: s0 + 2 * NB],
                                start=(m == 0),
                                stop=(m == mc[c] - 1),
                            )
                        off += mc[c]
                    # exact GELU while draining PSUM; tty is CHANNEL-major
                    # (columns c*256 + 2t + h) so this write is contiguous -
                    # strided engine writes run ~4x slower
                    nc.scalar.activation(
                        out=tty[:, k * 512: (k + 1) * 512],
                        in_=ps.rearrange("p a b h -> p (a b h)"),
                        func=GELU,
                    )

                    # emit the back-transpose of pair k-2 here: two pairs of
                    # delay keep the PE queue from head-of-line blocking on
                    # that pair's gelu drain
                    if k >= 2:
                        emit_bt(2 * (k - 2))
                emit_bt(C - 4)
                emit_bt(C - 2)

            if reps == 1:
                emit_body()
            else:
                # in-NEFF rep loop for delta timing: constant instruction
                # count, so huge rep counts stay cheap to compile
                with tc.For_i(0, reps):
                    emit_body()
            nc.sync.dma_start(out=y_out[:], in_=ynat[:])
    return nc


_NC_CACHE = {}


def _get_nc(mc, reps=1):
    key = (mc, reps)
    if key not in _NC_CACHE:
        _NC_CACHE[key] = _build_nc(mc, reps)
    return _NC_CACHE[key]


def _band_matrices(F, mc):
    """[P, (sum(mc)+1)*128] bf16: per-channel Toeplitz bands + identity."""
    na = sum(mc) + 1
    am = np.zeros((P, na * 128), np.float32)
    q = np.arange(128)
    off = 0
    for c in range(C):
        Fz = np.zeros(127 + 128 * MAXM + 128)
        Fz[127: 127 + FLEN] = F[c]
        win = np.lib.stride_tricks.sliding_window_view(Fz, 128)
        for m in range(mc[c]):
            # A[q, p] = F[c, p - q + 128 m]
            am[:, (off + m) * 128: (off + m + 1) * 128] = win[127 + 128 * m - q]
        off += mc[c]
    am[:, (na - 1) * 128: na * 128] = np.eye(128, dtype=np.float32)
    return am.astype(ml_dtypes.bfloat16)


def pack_inputs(x, h0, h1, w):
    F = _combined_filter(h0, h1, w)
    mc = _choose_mc(F)
    mh = max(mc) - 1
    am = _band_matrices(F, mc)

    in_maps = []
    for bi in range(NCORES):
        xr = np.ascontiguousarray(x[bi]).reshape(C, NH, NB, 128)
        # channel-major, (t, h)-interleaved columns with per-channel halo:
        # xt[p, c*cw + 2*(mh + t) + h] = x[bi, c, 16384*h + 128*t + p]
        full = np.zeros((P, C, mh + NB, NH), np.float32)
        full[:, :, mh:, :] = xr.transpose(3, 0, 2, 1)
        # half 1's causal history is half 0's last mh blocks
        full[:, :, :mh, 1] = xr[:, 0, NB - mh:, :].transpose(2, 0, 1)
        xt = full.reshape(P, C * (mh + NB) * NH).astype(ml_dtypes.bfloat16)
        in_maps.append({"xt": xt, "am": am})
    return in_maps, mc


def unpack_outputs(results):
    # device rows/cols are the 32x32 StreamTranspose of the channel-major
    # gelu buffer: y[32i + a, 32j + w] = gelu_y[c, 16384h + 128t + 32i + w]
    # with a = 2*a2 + h, j = 8c + jr, t = 16*jr + a2
    out = np.empty((B, C, L), np.float32)
    for bi, r in enumerate(results):
        yv = np.asarray(r["y"]).astype(np.float32)
        v = yv.reshape(4, 16, 2, C, 8, 32)           # [i, a2, h, c, jr, w]
        out[bi] = v.transpose(3, 2, 4, 1, 0, 5).reshape(C, L)
    return out


def kernel(x, h0, h1, w, _trace=False):
    import os
    os.environ.setdefault("BASS_NEVER_TRACE", "1")

    x = np.asarray(x, np.float32)
    h0 = np.asarray(h0, np.float32)
    h1 = np.asarray(h1, np.float32)
    w = np.asarray(w, np.float32)

    in_maps, mc = pack_inputs(x, h0, h1, w)
    nc = _get_nc(mc, 1)
    try:
        res = run_bass_kernel_spmd(
            nc, in_maps, core_ids=list(range(NCORES)), trace=_trace,
        )
    except Exception:
        # transient "device unrecoverable" failures have been observed on
        # this fleet; one retry usually succeeds
        res = run_bass_kernel_spmd(
            nc, in_maps, core_ids=list(range(NCORES)), trace=_trace,
        )
    out = unpack_outputs(res.results)
    if _trace:
        return out, res
    return out


# revision 34
# speedup vs baseline: 1.3068x; 1.0849x over previous
"""Trainium2 Bass kernel for nn_CausalMultiresConv1d.

Everything before the final GELU is linear: the whole multires stack is
one combined causal FIR filter per channel, F[c, 0:766], computed on the
host as the impulse response of the reference's linear part.

    out[b, c, n] = gelu( sum_tau F[c, tau] * x[b, c, n - tau] )

Sharding: pure data parallel - 1 batch element per NeuronCore (B=8).

Per-core algorithm (transposed layout so the conv is a PE matmul):
  xt[p, c*cw + 2*(mh + t) + h] = x[c, 16384*h + 128*t + p]        (host)
  i.e. positions-within-block on partitions; channel-major columns with
  (block t, half h) interleaved and mh private halo block-pairs per
  channel (zeros for half 0, the tail of half 0 for half 1), so every
  matmul rhs is one CONTIGUOUS window - strided PE/ACT access patterns
  measured ~3-4x slower.

  For each channel, the FIR becomes M_c banded matmuls accumulated in
  PSUM:   Y_c[p, (t,h)] = sum_m A_m^c.T @ xt[:, window shifted by m]
  with A_m^c[q, p] = F[c, p - q + 128 m]  (128x128 Toeplitz bands, bf16).
  M_c is per-channel via po-averaged tail energy (total truncation error
  ~4e-3 relative incl. bf16, tolerance is 2e-2).

  ACT drains PSUM with exact GELU (contiguous, channel-major); PE
  back-transposes each [128,128] tile (pipelined 2 channels behind the
  conv); DVE drains the transposed tiles bitcast-as-fp32; one DMA ships
  the bf16 result, and the host upcasts + un-interleaves rows.
"""

import numpy as np
import ml_dtypes

import concourse.bass as bass
import concourse.mybir as mybir
from concourse.bass_utils import run_bass_kernel_spmd
from concourse.tile import TileContext

# The walrus build here rejects instructions carrying more than one sync-wait
# ("Too many sync wait commands"). Tile's kernel-tail drain attaches a wait for
# every outstanding semaphore to a single SP Drain. _TC splits them: hoist all
# but the last wait onto dedicated single-wait NOPs preceding the drain.


class _TC(TileContext):
    def __exit__(self, *a):
        r = super().__exit__(*a)
        _split_multi_waits(self.nc)
        return r


def _split_multi_waits(nc):
    n = 0
    for fn in nc.m.functions:
        for blk in fn.blocks:
            insts = getattr(blk, "instructions", None)
            if insts is None:
                continue
            new = []
            for inst in insts:
                si = getattr(inst, "sync_info", None)
                waits = list(si.on_wait) if si is not None and si.on_wait else []
                if len(waits) > 1:
                    for j, wcmd in enumerate(waits[:-1]):
                        nop = mybir.InstNoOp(
                            name=f"{inst.name}-hw{j}", engine=inst.engine
                        )
                        nop.sync_info = mybir.SyncInfo(
                            on_wait=[wcmd], on_update=[]
                        )
                        new.append(nop)
                        n += 1
                    inst.sync_info = mybir.SyncInfo(
                        on_wait=[waits[-1]], on_update=list(si.on_update)
                    )
                new.append(inst)
            blk.instructions[:] = new
    return n


B, C, L = 8, 64, 32768
K, DEPTH = 4, 8
NCORES = 8
NH = 2                  # L-halves packed side by side in the channel dim
HL = L // NH            # 16384 positions per half
NB = HL // 128          # 128 blocks of 128 positions per half
P = 128
FLEN = 766              # combined filter support
MAXM = 7                # max 128-tap bands (covers 766 taps)
TRUNC_THR = 1e-5        # per-channel tail energy cutoff (frac of total)
TSEG = 8                # output blocks per PSUM segment (one 2KB bank)

F32 = mybir.dt.float32
BF16 = mybir.dt.bfloat16


def _combined_filter(h0, h1, w):
    """Impulse response [C, FLEN] of the linear part, in float64."""
    h0d = h0[:, 0, :].astype(np.float64)
    h1d = h1[:, 0, :].astype(np.float64)
    wd = w.astype(np.float64)

    def dconv(r, h, d):
        out = np.zeros_like(r)
        for k in range(K):
            s = (K - 1 - k) * d
            out[:, s:] += h[:, k:k + 1] * r[:, :FLEN - s]
        return out

    r = np.zeros((C, FLEN))
    r[:, 0] = 1.0
    y = np.zeros((C, FLEN))
    d = 1
    for i in range(DEPTH, 0, -1):
        y += wd[:, i][:, None] * dconv(r, h1d, d)
        r = dconv(r, h0d, d)
        d *= 2
    y += wd[:, 0][:, None] * r
    y[:, 0] += wd[:, -1]
    return y


def _choose_mc(F):
    """Per-channel band count. With nb bands, output position po of a block
    sees taps <= po + 128*(nb-1), so the truncation error is the po-AVERAGED
    dropped tail energy; pick the smallest nb that makes it negligible."""
    E = F * F
    tot = E.sum()
    # suffix[t] = sum of E[c, t:]
    suf = np.zeros((C, FLEN + 1))
    suf[:, :FLEN] = E[:, ::-1].cumsum(axis=1)[:, ::-1]
    mc = []
    for c in range(C):
        nb = MAXM
        for M in range(1, MAXM):
            lo = 128 * (M - 1)
            idx = np.minimum(lo + 1 + np.arange(128), FLEN)
            if suf[c, idx].mean() <= TRUNC_THR * tot:
                nb = M
                break
        mc.append(nb)
    return tuple(mc)


def _build_nc(mc, reps=1):
    nc = bass.Bass()
    mh = max(mc) - 1                      # halo blocks
    cw = 2 * mh + 2 * NB                  # columns per channel (halo + data)
    xt_cols = C * cw
    na = sum(mc) + 1                      # band matrices + identity
    xt_in = nc.dram_tensor("xt", [P, xt_cols], BF16, kind="ExternalInput")
    am_in = nc.dram_tensor("am", [P, na * 128], BF16, kind="ExternalInput")
    y_out = nc.dram_tensor("y", [P, HL], BF16, kind="ExternalOutput")

    GELU = mybir.ActivationFunctionType.Gelu

    with _TC(nc) as tc:
        with (
            tc.tile_pool(name="main", bufs=1) as pool,
            tc.tile_pool(name="psum", bufs=1, space="PSUM") as psum_pool,
        ):
            xts = pool.tile([P, xt_cols], BF16, tag="xts")
            ams = pool.tile([P, na * 128], BF16, tag="ams")
            tty = pool.tile([P, NB * 128], BF16, tag="tty")
            ynat = pool.tile([P, NB * 128], BF16, tag="ynat")

            nc.sync.dma_start(out=xts[:], in_=xt_in[:])
            nc.sync.dma_start(out=ams[:], in_=am_in[:])

            def emit_bt(c0):
                # back-transpose channels c0, c0+1 with DVE StreamTranspose
                # (SBUF->SBUF, 32x32 blocks transposed in place-position).
                # This keeps the whole reorder off the PE and skips the
                # PSUM round-trip; the host unpack absorbs the block-grid
                # permutation (it is a pure numpy axis permute either way).
                src = c0 * 256
                nc.vector.transpose(
                    ynat[:, src: src + 512], tty[:, src: src + 512],
                )

            def emit_body():
                # conv: per channel, mc[c] banded matmuls accumulated in
                # PSUM. xt is channel-major with (t, h)-interleaved columns
                # and a private halo per channel, so every rhs is one
                # CONTIGUOUS 256-column window (strided rhs runs ~3x slower
                # on the PE's SBUF read path). Channels are processed in
                # pairs sharing one PSUM bank so each gelu drain covers 512
                # columns - ACT per-instruction overhead is a co-bottleneck.
                off = 0
                for k in range(C // 2):
                    ps = psum_pool.tile([P, 2, 128, 2], F32, tag="ps", bufs=4)
                    for ci in range(2):
                        c = 2 * k + ci
                        for m in range(mc[c]):
                            s0 = c * cw + 2 * (mh - m)
                            nc.tensor.matmul(
                                ps[:, ci],
                                lhsT=ams[:, (off + m) * 128:
                                          (off + m + 1) * 128],
                                rhs=xts[:, s0: s0 + 2 * NB],
                                start=(m == 0),
                                stop=(m == mc[c] - 1),
                            )
                        off += mc[c]
                    # exact GELU while draining PSUM; tty is CHANNEL-major
                    # (columns c*256 + 2t + h) so this write is contiguous -
                    # strided engine writes run ~4x slower
                    nc.scalar.activation(
                        out=tty[:, k * 512: (k + 1) * 512],
                        in_=ps.rearrange("p a b h -> p (a b h)"),
                        func=GELU,
                    )

                    # emit the back-transpose of pair k-2 here: two pairs of
                    # delay keep the PE queue from head-of-line blocking on
                    # that pair's gelu drain
                    if k >= 2:
                        emit_bt(2 * (k - 2))
                emit_bt(C - 4)
                emit_bt(C - 2)

            if reps == 1:
                emit_body()
            else:
                # in-NEFF rep loop for delta timing: constant instruction
                # count, so huge rep counts stay cheap to compile
                with tc.For_i(0, reps):
                    emit_body()
            nc.sync.dma_start(out=y_out[:], in_=ynat[:])
    return nc


_NC_CACHE = {}


def _get_nc(mc, reps=1):
    key = (mc, reps)
    if key not in _NC_CACHE:
        _NC_CACHE[key] = _build_nc(mc, reps)
    return _NC_CACHE[key]


def _band_matrices(F, mc):
    """[P, (sum(mc)+1)*128] bf16: per-channel Toeplitz bands + identity."""
    na = sum(mc) + 1
    am = np.zeros((P, na * 128), np.float32)
    q = np.arange(128)
    off = 0
    for c in range(C):
        Fz = np.zeros(127 + 128 * MAXM + 128)
        Fz[127: 127 + FLEN] = F[c]
        win = np.lib.stride_tricks.sliding_window_view(Fz, 128)
        for m in range(mc[c]):
            # A[q, p] = F[c, p - q + 128 m]
            am[:, (off + m) * 128: (off + m + 1) * 128] = win[127 + 128 * m - q]
        off += mc[c]
    am[:, (na - 1) * 128: na * 128] = np.eye(128, dtype=np.float32)
    return am.astype(ml_dtypes.bfloat16)


def pack_inputs(x, h0, h1, w):
    F = _combined_filter(h0, h1, w)
    mc = _choose_mc(F)
    mh = max(mc) - 1
    am = _band_matrices(F, mc)

    in_maps = []
    for bi in range(NCORES):
        xr = np.ascontiguousarray(x[bi]).reshape(C, NH, NB, 128)
        # channel-major, (t, h)-interleaved columns with per-channel halo:
        # xt[p, c*cw + 2*(mh + t) + h] = x[bi, c, 16384*h + 128*t + p]
        full = np.zeros((P, C, mh + NB, NH), np.float32)
        full[:, :, mh:, :] = xr.transpose(3, 0, 2, 1)
        # half 1's causal history is half 0's last mh blocks
        full[:, :, :mh, 1] = xr[:, 0, NB - mh:, :].transpose(2, 0, 1)
        xt = full.reshape(P, C * (mh + NB) * NH).astype(ml_dtypes.bfloat16)
        in_maps.append({"xt": xt, "am": am})
    return in_maps, mc


def unpack_outputs(results):
    # device rows/cols are the 32x32 StreamTranspose of the channel-major
    # gelu buffer: y[32i + a, 32j + w] = gelu_y[c, 16384h + 128t + 32i + w]
    # with a = 2*a2 + h, j = 8c + jr, t = 16*jr + a2
    out = np.empty((B, C, L), np.float32)
    for bi, r in enumerate(results):
        yv = np.asarray(r["y"]).astype(np.float32)
        v = yv.reshape(4, 16, 2, C, 8, 32)           # [i, a2, h, c, jr, w]
        out[bi] = v.transpose(3, 2, 4, 1, 0, 5).reshape(C, L)
    return out


def kernel(x, h0, h1, w, _trace=False):
    import os
    os.environ.setdefault("BASS_NEVER_TRACE", "1")

    x = np.asarray(x, np.float32)
    h0 = np.asarray(h0, np.float32)
    h1 = np.asarray(h1, np.float32)
    w = np.asarray(w, np.float32)

    in_maps, mc = pack_inputs(x, h0, h1, w)
    nc = _get_nc(mc, 1)
    try:
        res = run_bass_kernel_spmd(
            nc, in_maps, core_ids=list(range(NCORES)), trace=_trace,
        )
    except Exception:
        # transient "device unrecoverable" failures have been observed on
        # this fleet; one retry usually succeeds
        res = run_bass_kernel_spmd(
            nc, in_maps, core_ids=list(range(NCORES)), trace=_trace,
        )
    out = unpack_outputs(res.results)
    if _trace:
        return out, res
    return out
